# revision 33
# baseline (speedup 1.0000x reference)
"""Trainium2 Bass kernel for KNN OOD scoring (nn_KNNModel).

Computation (matches reference):
  queries = embeddings [B=4, D=128, 32, 32] -> 4096 per-pixel queries
  d(q, bank_i) euclidean over N=50000 bank rows, k=5 nearest,
  score = mean distance, bilinear upsample 32x32 -> 512x512.

Sharding: query-parallel over 8 cores (core c owns batch c//2, 16-row
band c%2). Each core resolves its 512 queries against the full bank.

Device algorithm (per core): the bank is padded to 52224 rows and split
into 26112 (a, b) pairs. For each query block (128 queries stationary)
and each chunk of 1536 pairs, PSUM accumulates
    D = 2q.(a-b) - (|a|^2-|b|^2)        (2 fp8 DoubleRow matmuls: hi+lo)
then ScalarE applies Relu in place, then a third DoubleRow matmul
accumulates B = 2q.b - |b|^2 on top, yielding max(vA, vB) per pair
(v = 2q.x - |x|^2; max v  <=>  min distance). DVE max8 extracts the 8
largest per chunk; a final max8-of-candidates gives the global top-5,
and d = sqrt(|q|^2 - v) with a fused accumulate gives the score sum.

fp8 quantization: bank features are stored hi+lo e4m3 (the lo matmul
rides a zero-filled second k-subtile); squared-feature rows are
residual-carry quantized so their per-column sums are nearly exact.

Query scores for the band's seam row are computed in qtile 0 (queries
are permuted so the seam row comes first); a 4KB AllGather then runs
concurrently with the remaining query blocks, and the bilinear resize
mixes the local 16 rows plus the partner core's seam row via a
host-built [24, 256] vertical weight matrix (zero columns select the
partner), so the SPMD program has no core-dependent addressing.
"""

import os
import time

import numpy as np
import ml_dtypes

import concourse.bass as bass
from concourse import bacc
import concourse.mybir as mybir
import concourse.tile as tile
from concourse.bass_utils import run_bass_kernel_spmd

# ---- problem constants (hardcoded per contract) ----
B, D, H, W = 4, 128, 32, 32
N_BANK = 50000
K_NN = 5
OUT_H = OUT_W = 512

NPAD = 50000                # bank rows (no padding needed)
TPAIR = NPAD // 2           # 25000 pairs
CHUNK = 1000                # pair-columns per chunk
NCHUNKS = TPAIR // CHUNK    # 25
BAND_ROWS = 16
QPC = BAND_ROWS * W         # 512 queries per core
QTILES = 4
PAD_VAL = 15.0

E4 = ml_dtypes.float8_e4m3

LAST_EXEC_NS = None


def _resize_weight(out_size, in_size):
    """jax.image.resize(method='bilinear') triangle-kernel weights."""
    scale = out_size / in_size
    sample_f = (np.arange(out_size) + 0.5) / scale - 0.5
    x = np.abs(sample_f[:, None] - np.arange(in_size)[None, :])
    w = np.maximum(0.0, 1.0 - x)
    w = w / w.sum(axis=1, keepdims=True)
    return w.astype(np.float32)  # [out, in]


def build_kernel():
    """Build the per-core SPMD Bass program. Returns compiled nc."""
    nc = bacc.Bacc("TRN2", target_bir_lowering=False)
    f32 = mybir.dt.float32
    fp8 = mybir.dt.float8e4
    DR = mybir.MatmulPerfMode.DoubleRow

    xd_d = nc.dram_tensor("xd", [D, NCHUNKS, 2, CHUNK], fp8, kind="ExternalInput")
    xl_d = nc.dram_tensor("xl", [D, NCHUNKS, 2, CHUNK], fp8, kind="ExternalInput")
    xb_d = nc.dram_tensor("xb", [D, NCHUNKS, 2, CHUNK], fp8, kind="ExternalInput")
    qw_d = nc.dram_tensor("qw", [D, 2, QPC], fp8, kind="ExternalInput")
    q2_d = nc.dram_tensor("q2", [128, QTILES], f32, kind="ExternalInput")
    f32r = mybir.dt.float32r
    wh_d = nc.dram_tensor("wh", [W, OUT_W], f32r, kind="ExternalInput")
    wv_d = nc.dram_tensor("wv", [24, 256], f32r, kind="ExternalInput")
    whr_d = nc.dram_tensor("whr", [128, OUT_W], f32r, kind="ExternalInput")
    wq3_d = nc.dram_tensor("wq3", [128, 256], f32r, kind="ExternalInput")
    out_d = nc.dram_tensor("out", [256, OUT_W], f32, kind="ExternalOutput")
    s0_d = nc.dram_tensor("s0", [128], f32r)
    sr_d = nc.dram_tensor("sr", [3, 128], f32r)
    scall_d = nc.dram_tensor("scall", [8 * 128], f32r, addr_space="Shared")

    with tile.TileContext(nc) as tc:
        with (
            tc.tile_pool(name="sb", bufs=2) as sb,
            tc.tile_pool(name="pers", bufs=1) as pers,
            tc.tile_pool(name="ps", bufs=4, space="PSUM") as ps,
        ):
            # stationary queries first (needed by the first chains)
            qw = pers.tile([D, 2, QPC], fp8, tag="qw")
            nc.sync.dma_start(out=qw[:], in_=qw_d.ap())

            # resident fp8 bank matrices, one tile per chunk for precise deps
            CH2G = {c: (c, 0) for c in range(NCHUNKS)}
            queues = [nc.sync, nc.scalar, nc.sync]
            xd, xl, xb = [], [], []
            for c in range(NCHUNKS):
                for qi, (lst, dram, nm) in enumerate(
                        ((xd, xd_d, "xd"), (xl, xl_d, "xl"), (xb, xb_d, "xb"))):
                    tl = pers.tile([D, 1, 2, CHUNK], fp8, tag=f"{nm}{c}",
                                   name=f"{nm}{c}")
                    eng = queues[qi] if c < 2 else nc.sync
                    eng.dma_start(out=tl[:], in_=dram.ap()[:, c:c + 1])
                    lst.append(tl)

            # tail-only constants: issue behind the bank stream
            q2 = pers.tile([128, QTILES], f32, tag="q2")
            nc.sync.dma_start(out=q2[:], in_=q2_d.ap())
            wh = pers.tile([W, OUT_W], f32r, tag="wh")
            nc.sync.dma_start(out=wh[:], in_=wh_d.ap())
            wv = pers.tile([24, 256], f32r, tag="wv")
            nc.sync.dma_start(out=wv[:], in_=wv_d.ap())
            whr = pers.tile([128, OUT_W], f32r, tag="whr")
            nc.sync.dma_start(out=whr[:], in_=whr_d.ap())
            wq3 = pers.tile([128, 256], f32r, tag="wq3")
            nc.sync.dma_start(out=wq3[:], in_=wq3_d.ap())

            cand = [
                pers.tile([128, 8 * NCHUNKS], f32, tag=f"cand{t}",
                          name=f"cand{t}")
                for t in range(QTILES)
            ]

            def chain(t, c):
                """One chunk of 1536 pair-maxes for query tile t."""
                lhs = qw[:, :, t * 128:(t + 1) * 128]
                P = ps.tile([128, CHUNK], f32, tag="chain")
                g, ci = CH2G[c]
                for s in range(2):
                    sl = slice(s * 512, min((s + 1) * 512, CHUNK))
                    nc.tensor.matmul(out=P[:, sl], lhsT=lhs, rhs=xd[g][:, ci, :, sl],
                                     start=True, stop=False, perf_mode=DR)
                    nc.tensor.matmul(out=P[:, sl], lhsT=lhs, rhs=xl[g][:, ci, :, sl],
                                     start=False, stop=True, perf_mode=DR)
                nc.scalar.activation(P[:], P[:], mybir.ActivationFunctionType.Relu)
                for s in range(2):
                    sl = slice(s * 512, min((s + 1) * 512, CHUNK))
                    nc.tensor.matmul(out=P[:, sl], lhsT=lhs, rhs=xb[g][:, ci, :, sl],
                                     start=False, stop=True, perf_mode=DR,
                                     skip_group_check=True)
                nc.vector.max(cand[t][:, 8 * c:8 * (c + 1)], P[:])

            def qtail(t):
                """Global top-5 -> summed distances -> score sink for tile t."""
                top8 = sb.tile([128, 8], f32, tag="top8")
                nc.vector.max(top8[:], cand[t][:])
                d5 = sb.tile([128, K_NN], f32, tag="d5")
                ssum = sb.tile([128, 1], f32, tag="ssum")
                nc.scalar.activation(
                    d5[:], top8[:, 0:K_NN], mybir.ActivationFunctionType.Sqrt,
                    scale=-1.0, bias=q2[:, t:t + 1], accum_out=ssum[:],
                )
                if t == 0:
                    nc.sync.dma_start(out=s0_d.ap().bitcast(f32),
                                      in_=ssum[:].rearrange("p one -> (p one)"))
                elif t < 3:
                    nc.sync.dma_start(out=sr_d.ap()[t - 1].bitcast(f32), in_=ssum[:])
                return ssum

            # pass A: qtiles 0/1/2 interleaved behind the DMA stream (compute
            # demand ~3.6us/chunk far exceeds DMA supply ~2.2us/chunk, so the
            # engines never starve); qtile 3 then covers the collective.
            for c in range(NCHUNKS):
                chain(0, c)
                chain(1, c)
                chain(2, c)
            qtail(0)
            # 4KB all-gather of qtile-0 scores, overlapped with qtile 3
            nc.gpsimd.collective_compute(
                "AllGather",
                mybir.AluOpType.bypass,
                replica_groups=[[0, 1, 2, 3, 4, 5, 6, 7]],
                ins=[s0_d.ap()],
                outs=[scall_d.ap()],
            )
            # s_t tile created before qtails so qtail(3) can write into it
            s_t = sb.tile([W, 24], f32r, tag="s_t")
            qtail(1)
            qtail(2)
            # s_t columns that do not depend on qtile 3: assembled during the
            # qtile-3 pass (the seam columns wait on the collective).
            nc.sync.dma_start(out=s_t[:, 0:4],
                              in_=s0_d.ap().rearrange("(s w) -> w s", w=W))
            nc.sync.dma_start(out=s_t[:, 4:12],
                              in_=sr_d.ap()[0:2].rearrange("s p -> (s p)").rearrange("(s w) -> w s", w=W))
            nc.sync.dma_start(out=s_t[:, 16:24],
                              in_=scall_d.ap().rearrange("(c x) -> x c", x=128)[0:W])
            for c in range(NCHUNKS):
                chain(3, c)

            # ---- bilinear resize tail ----
            # s_t [32, 24]: cols 0..15 local scores in slot order, cols
            # 16..23 every core's qtile-0 seam row (host weights pick the
            # partner and zero the rest).

            # qtile 0-2 + seam part of the resize (wv rows 12:16 are zero on
            # the host; s_t cols 12:16 are zero-filled below)
            nc.vector.memset(s_t[:, 12:16].bitcast(f32), 0.0)
            t1p = ps.tile([128, CHUNK], f32, tag="chain")
            nc.tensor.matmul(out=t1p[0:24, 0:OUT_W], lhsT=s_t[:], rhs=wh[:],
                             start=True, stop=True)
            t1 = sb.tile([24, OUT_W], f32r, tag="t1")
            nc.scalar.activation(t1[:], t1p[0:24, 0:OUT_W],
                                 mybir.ActivationFunctionType.Copy)
            ops = []
            for hh in range(2):
                op = ps.tile([128, CHUNK], f32, tag="chain")
                nc.tensor.matmul(
                    out=op[:, 0:OUT_W], lhsT=wv[:, hh * 128:(hh + 1) * 128],
                    rhs=t1[:], start=True, stop=False,
                )
                ops.append(op)

            # qtile 3 folded in algebraically: out += wq3^T @ (whr * ssum3)
            ssum3 = qtail(3)
            bmat = sb.tile([128, OUT_W], f32r, tag="bmat")
            nc.scalar.activation(bmat[:], whr[:],
                                 mybir.ActivationFunctionType.Copy,
                                 scale=ssum3[:])
            for hh in range(2):
                nc.tensor.matmul(
                    out=ops[hh][:, 0:OUT_W],
                    lhsT=wq3[:, hh * 128:(hh + 1) * 128], rhs=bmat[:],
                    start=False, stop=True, skip_group_check=True,
                )
                o_sb = sb.tile([128, OUT_W], f32, tag="o_sb")
                nc.scalar.activation(o_sb[:], ops[hh][:, 0:OUT_W],
                                     mybir.ActivationFunctionType.Copy)
                nc.sync.dma_start(out=out_d.ap()[hh * 128:(hh + 1) * 128, :],
                                  in_=o_sb[:])

    nc.compile()
    return nc


def _carry_quant_neg(x):
    """e4m3 quantization of -x along axis 0 with residual carry, so the
    per-column sums of the fp8 values track the exact sums closely."""
    out = np.empty(x.shape, E4)
    r = np.zeros(x.shape[1], np.float32)
    for d in range(x.shape[0]):
        t = -x[d] + r
        q = t.astype(E4)
        out[d] = q
        r = t - q.astype(np.float32)
    return out


_BANK_CACHE = {}


def _prep_bank(bank):
    """Shared (all cores) fp8 bank matrices [D, NCHUNKS, 2, CHUNK]."""
    key = bank.ctypes.data
    if _BANK_CACHE.get("key") == key:
        return _BANK_CACHE["val"]
    a, b = bank[:TPAIR], bank[TPAIR:]

    diff = a - b
    dhi = diff.astype(E4)
    dlo = (diff - dhi.astype(np.float32)).astype(E4)
    bhi = b.astype(E4)
    nsqd = _carry_quant_neg((a * a - b * b).T)    # [D, TPAIR]
    nsqb = _carry_quant_neg((b * b).T)

    def pack(feat, sq):
        # feat [TPAIR, D] fp8, sq [D, TPAIR] fp8 -> [D, NCHUNKS, 2, CHUNK]
        out = np.empty([D, NCHUNKS, 2, CHUNK], E4)
        out[:, :, 0, :] = feat.T.reshape(D, NCHUNKS, CHUNK)
        out[:, :, 1, :] = sq.reshape(D, NCHUNKS, CHUNK)
        return out

    zeros = np.zeros([D, TPAIR], E4)
    val = (pack(dhi, nsqd), pack(dlo, zeros), pack(bhi, nsqb))
    _BANK_CACHE["key"] = key
    _BANK_CACHE["val"] = val
    return val


def make_in_maps(embeddings, bank):
    """Host-side shard prep: per-core input dict."""
    xd, xl, xb = _prep_bank(bank)

    wh_full = _resize_weight(OUT_W, W)            # [512, 32]
    wv_full = _resize_weight(OUT_H, H)            # [512, 32]
    wh = np.ascontiguousarray(wh_full.T) * (1.0 / K_NN)  # [32, 512]

    in_maps = []
    for c in range(8):
        bidx, band = c // 2, c % 2
        if band == 0:
            rows = [15] + list(range(0, 15))      # seam row first
            partner_seam = 16
        else:
            rows = [16] + list(range(17, 32))
            partner_seam = 15
        q = embeddings[bidx][:, rows, :].reshape(D, QPC)   # [D, 512] slot order
        qw = np.empty([D, 2, QPC], E4)
        qw[:, 0, :] = (2.0 * q).astype(E4)
        qw[:, 1, :] = np.float32(1.0)
        q2 = (q * q).sum(axis=0).astype(np.float32)        # [512]
        q2m = q2.reshape(QTILES, 128).T.copy()             # [128, 4]

        wv_band = wv_full[band * 256:(band + 1) * 256]     # [256, 32]
        wv_ext = np.zeros([24, 256], np.float32)
        for s, nat in enumerate(rows):
            wv_ext[s] = wv_band[:, nat]
        wv_ext[16 + (c ^ 1)] = wv_band[:, partner_seam]
        # qtile 3 (slots 12..15) enters via the rank update, not via s_t
        wq3 = np.ascontiguousarray(
            np.repeat(wv_ext[12:16], W, axis=0))           # [128, 256]
        wv_ext = wv_ext.copy()
        wv_ext[12:16] = 0.0
        whr = np.ascontiguousarray(np.tile(wh, (QTILES, 1)))  # [128, 512]

        in_maps.append({
            "xd": xd, "xl": xl, "xb": xb,
            "qw": qw, "q2": q2m,
            "wh": wh, "wv": wv_ext, "whr": whr, "wq3": wq3,
        })
    return in_maps


_NC_CACHE = {}


def kernel(embeddings, bank, k, out_h, out_w):
    global LAST_EXEC_NS
    embeddings = np.asarray(embeddings, dtype=np.float32)
    bank = np.asarray(bank, dtype=np.float32)
    assert int(k) == K_NN and int(out_h) == OUT_H and int(out_w) == OUT_W
    assert embeddings.shape == (B, D, H, W) and bank.shape == (N_BANK, D)

    if "nc" not in _NC_CACHE:
        _NC_CACHE["nc"] = build_kernel()
    nc = _NC_CACHE["nc"]

    in_maps = make_in_maps(embeddings, bank)
    trace = bool(int(os.environ.get("KNN_TRACE", "0")))
    t0 = time.time()
    res = run_bass_kernel_spmd(nc, in_maps, list(range(8)), trace=trace)
    t1 = time.time()
    LAST_EXEC_NS = res.exec_time_ns if res.exec_time_ns else int((t1 - t0) * 1e9)

    full = np.zeros([B, 1, OUT_H, OUT_W], dtype=np.float32)
    for c in range(8):
        bidx, band = c // 2, c % 2
        full[bidx, 0, band * 256:(band + 1) * 256, :] = res.results[c]["out"]
    return full
